# revision 23
# baseline (speedup 1.0000x reference)
"""Average-pool (window 4, non-overlapping) over last dim of x:(128,4,65536) f32.

Sharding: pure data parallel — batch dim 128 split into 8 shards of 16.
Each core's shard (16*4*65536 = 4,194,304 f32) is viewed as [128, 32768]
(partition-major); 32768 % 4 == 0 so window boundaries are preserved per
partition row. Per-core output is [128, 8192].

Per-core pipeline (streams at the per-core HBM share, ~370 GB/s):
  DMA in [128, W] (sync HWDGE ring) -> DVE tensor_reduce(axis=X) over
  [128, W/4, 4] -> ACT in-place scale by 0.25 -> DMA out [128, W/4]
  (scalar HWDGE ring, ordered after the scale by engine program order)

Walrus codegen limits shape this kernel (the axon/bass2jax path
compiles BIR through walrus, unlike the native bench): an instruction
encodes very few sync waits (a DMACopy exactly ONE, the tail Drain <5).
So:
  * no SBUF slot reuse by DMAs (every tile gets its own slot via
    distinct tags) -> loads carry 0 waits, stores at most 1;
  * at most 8 DMAs total so no HWDGE sem lane is reused (a reused lane
    puts an increment-ordering wait on the later DMA);
  * TileContext._drain_and_barrier is patched to pre-split the tail
    drain's wait list into single-wait SP NOPs.
"""

import sys
import types

import numpy as np

import concourse.bass as bass
import concourse.tile as tile
from concourse import mybir
from concourse.bass_utils import run_bass_kernel_spmd
from concourse.vector_clock import ScopedClock


def _ensure_ntff_hook_module():
    """The agent image's `antenv` stub lacks `axon_hooks`; bass_utils
    imports it whenever tracing is requested (e.g. BASS_TRACE=1) and
    would crash. Provide the module, backed by the ctypes NTFF driver
    when available, else a no-hook fallback."""
    if "antenv.axon_hooks" in sys.modules:
        return
    try:
        import antenv.axon_hooks  # noqa: F401
        return
    except ImportError:
        pass
    hook = None
    try:
        from trn_agent_boot.trn_boot import _ntff_profile_via_ctypes
        hook = _ntff_profile_via_ctypes("/opt/axon/libaxon_pjrt.so")
    except Exception:
        pass
    mod = types.ModuleType("antenv.axon_hooks")
    mod.get_axon_ntff_profile_hook = lambda: hook
    mod.set_axon_ntff_profile_hook = lambda h: None
    sys.modules["antenv.axon_hooks"] = mod


_ensure_ntff_hook_module()

N_CORES = 8
P = 128
F_TOT = 32768          # free elems per partition per core = 16*4*65536/128
SCALE = 4
G_TOT = F_TOT // SCALE

# Per-tile free widths; sum == F_TOT. Tapered so the last tile's
# reduce+store tail after the final load is short.
WIDTHS = (11264, 11264, 8192, 2048)


def _split_wait_drain_and_barrier(self, tick_clock, wait_clock):
    """Replacement for TileContext._drain_and_barrier:
    * outstanding sem waits are emitted as single-wait SP NOPs before
      the drain (walrus can't encode a multi-wait Drain);
    * only store-DMA completion sems are waited on — every other sem's
      final value is transitively implied by them (stores wait on ACT,
      ACT on DVE, DVE consumed each load's completion sem).
    """
    nc = self.nc
    probe = mybir.InstNoOp(name=nc.get_next_instruction_name(),
                           engine=mybir.EngineType.SP)
    wait_clock.add_sem_waits(probe, ScopedClock({None: tick_clock.global_clock}))
    keep = None
    store_insts = getattr(nc, "_store_dma_insts", None)
    if store_insts:
        keep = set()
        for bi in store_insts:
            si = bi.ins.sync_info
            for u in (si.on_update if si is not None else []):
                keep.add((u.sync_type, u.id))
    if probe.sync_info is not None:
        for w in probe.sync_info.on_wait:
            if keep is not None and (w.sync_type, w.id) not in keep:
                continue
            n = nc.sync.nop(nofuse=True)
            n.ins.sync_info = mybir.SyncInfo(on_wait=[w], on_update=[])
    nc.sync.drain()
    nc.all_engine_barrier()
    assert self.sems is not None
    popped = nc._tile_sem_poison_stack.pop()
    assert popped is self._sem_poison
    nc.clear_and_free_semaphores(list(self.sems.allocated().values()))
    nc.all_engine_barrier()


tile.TileContext._drain_and_barrier = _split_wait_drain_and_barrier


_orig_memset = bass.BassEitherVectorEngine.memset


def _memset_skip_consts(self, ap, constant):
    # Skip the Bass preamble's four const-tile uploads ([128,1] each):
    # this kernel never reads them (scalars are instruction immediates)
    # and their Q7 memsets sit on the preamble critical path.
    nm = getattr(ap, "name", "") or ""
    if isinstance(nm, str) and nm.startswith("const-"):
        return None
    return _orig_memset(self, ap, constant)


def _build(widths=WIDTHS):
    bass.BassEitherVectorEngine.memset = _memset_skip_consts
    try:
        nc = bass.Bass("TRN2", target_bir_lowering=False, debug=False,
                       num_devices=N_CORES, enable_partition_id=False)
    finally:
        bass.BassEitherVectorEngine.memset = _orig_memset
    x = nc.dram_tensor("x", [P, F_TOT], mybir.dt.float32,
                       kind="ExternalInput").ap()
    y = nc.dram_tensor("y", [P, G_TOT], mybir.dt.float32,
                       kind="ExternalOutput").ap()
    assert sum(widths) == F_TOT
    with tile.TileContext(nc) as tc:
        with tc.tile_pool(name="inp", bufs=1) as inp, \
             tc.tile_pool(name="red", bufs=1) as redp:
            xo = 0
            yo = 0
            for i, w in enumerate(widths):
                g = w // SCALE
                t = inp.tile([P, w], mybir.dt.float32, tag=f"in{i}")
                nc.sync.dma_start(out=t[:], in_=x[:, xo:xo + w])
                r = redp.tile([P, g], mybir.dt.float32, tag=f"r{i}")
                nc.vector.tensor_reduce(
                    out=r[:],
                    in_=t[:].rearrange("p (g s) -> p g s", s=SCALE),
                    axis=mybir.AxisListType.X,
                    op=mybir.AluOpType.add,
                )
                nc.scalar.mul(r[:], r[:], 1.0 / SCALE)
                st = nc.scalar.dma_start(out=y[:, yo:yo + g], in_=r[:])
                nc._store_dma_insts = getattr(nc, "_store_dma_insts", []) + [st]
                xo += w
                yo += g
    return nc


_NC = None


def _get_nc():
    global _NC
    if _NC is None:
        _NC = _build()
    return _NC


def _run(x: np.ndarray, **kw):
    """Shard, run on 8 cores, gather. Returns (out, BassKernelResults)."""
    n, c, L = x.shape
    shards = np.ascontiguousarray(x, dtype=np.float32).reshape(N_CORES, P, F_TOT)
    in_maps = [{"x": shards[i]} for i in range(N_CORES)]
    res = run_bass_kernel_spmd(_get_nc(), in_maps, list(range(N_CORES)), **kw)
    out = np.stack([res.results[i]["y"] for i in range(N_CORES)])
    return out.reshape(n, c, L // SCALE), res


_WARMED = False


def kernel(x: np.ndarray) -> np.ndarray:
    global _WARMED
    if not _WARMED:
        _WARMED = True
        _run(x)  # warm-up execution: first run is ~10% slower (cold HBM/power)
    out, _ = _run(x)
    return out
